# revision 30
# baseline (speedup 1.0000x reference)
"""Trainium2 Bass kernel for nn_BinConv2d: BN(train-mode) -> sign -> 3x3 conv.

ONE fused launch on 8 cores, batch-sharded (2 images/core, 128
partitions = 2 img x 64 ch), x read once as fp16 into a resident SBUF
slab (12.85MB/core):

  Phase 1 (stats): x16 streams into the resident slab in 10 chunks;
    DVE bn_stats consumes 66 of the 98 512-col groups, ACT Copy/Square
    with accum_out the other 32 (per-<=1024 sub-groups).  Per-partition
    sums are combined on-device (f32), folded across the two image
    halves, AllReduced across the 8 cores (512B DRAM collective), and
    turned into the per-channel sign threshold
    t_c = mean_c - (bias_c/gamma_c)*sqrt(var_c+eps); tneg = -t lands in
    a [128,1] tile that phase 2's Sign bias port reads (fp16 x costs
    ~5e-3 rel err from sign flips at the threshold; measured 341 flips
    of 51.4M elements on the reference inputs).

  Phase 2 (conv): per band (28 output rows), sign(x) runs 128
    partitions wide on ACT straight out of the resident slab into a tmp
    tile, then 4 SBUF->SBUF DMA copies (scalar queue for strip A,
    gpsimd for strip B) build two per-image strips in fp8e4 ({-1,0,1}
    exact): partitions = 64 ch x 2 halves, second half shifted up one
    row-slot, so an AP strip[:, 2k*226+dx] yields rows 2k/2k+1 across
    the halves.  Matmuls are double-tile: one instruction computes TWO
    2-row tiles (moving AP [2, 224] with slot-pair stride), free size
    448, psum tile [128, 2, 224] = one 2KB bank; 6 matmuls (2 row-pairs
    x 3 dx, fp16 weights x fp8 strip) accumulate a 4-row tile; 7 tiles
    per image per band, in 4+3 half-band chunks so two chunks share the
    8 psum banks and DVE evacuations (+bias, 448 wide, fp16 out)
    overlap the next chunk's matmuls.  Sign runs one band ahead of the
    matmuls (double-buffered strips, 6 tmp buffers) so copy latency is
    off the critical path.  y is written fp16 by the sync queue (idle
    after phase 1) in the parity-split layout [128, 2, 112, 224] and
    unshuffled + upcast on host.
"""

import sys

if "/opt/trn_rl_repo" not in sys.path:
    sys.path.insert(0, "/opt/trn_rl_repo")

import numpy as np

import concourse.bacc as bacc
import concourse.tile as tile
from concourse import mybir
from concourse.bass_utils import run_bass_kernel_spmd

F32 = mybir.dt.float32
F16 = mybir.dt.float16
F8 = mybir.dt.float8e4

N_CORES = 8
N, C, H, W = 16, 64, 224, 224
BN_EPS = 1e-4
BAND = 28              # output rows per band
NB = H // BAND         # 8 bands
WP = W + 2             # padded strip width (226)
NT = BAND // 2         # 14 2-row tiles per band
NU = BAND // 4         # 7 4-row (double) tiles per band
SLOTS = BAND + 2       # 30 strip slots per band
STRIP_LEN = SLOTS * WP
HH = H // 2            # 112
NTOT = N * H * W       # BN sample count per channel

# stats chunking: (n_groups, engine); 'A' chunks go to ACT (sum/sum^2 via
# accum_out), 'D' chunks to DVE bn_stats.  ACT chunks early, small DVE
# chunk last to shorten the post-DMA tail.
STATS_CHUNKS = [(2, "D"), (13, "A"), (8, "D"), (13, "A"), (13, "D"),
                (6, "A"), (13, "D"), (13, "D"), (13, "D"), (4, "D")]
N_DVE_GROUPS = sum(g for g, e in STATS_CHUNKS if e == "D")  # 66
NAC = sum((g * 512 + 1023) // 1024 for g, e in STATS_CHUNKS if e == "A")  # 17


def build_fused_nc(repeat=1):
    """Single-launch kernel: x16 [128, H*W] f16, wts [128, 12, 128] f16
    lhsT bank, bgi [128,1] f32 (bn_bias/bn_weight), cbias [128,1] f32
    -> y [128, 2, 112, 224] f16 (parity-split device layout)."""
    nc = bacc.Bacc()
    cols = H * W
    x16 = nc.declare_dram_parameter("x16", [128, cols], F16, isOutput=False)
    wts = nc.declare_dram_parameter("wts", [128, 12, 128], F16, isOutput=False)
    bgi = nc.declare_dram_parameter("bgi", [128, 1], F32, isOutput=False)
    cbias = nc.declare_dram_parameter("cbias", [128, 1], F32, isOutput=False)
    y = nc.declare_dram_parameter("y", [128, 2, HH, W], F16, isOutput=True)
    cc_in = nc.dram_tensor("cc_in", [64, 2], F32)
    cc_out = nc.dram_tensor("cc_out", [64, 2], F32)
    cc_win = nc.dram_tensor("cc_win", [64, 2], F32)
    cc_wout = nc.dram_tensor("cc_wout", [64, 2], F32)

    assert sum(g for g, _ in STATS_CHUNKS) == cols // 512

    with tile.TileContext(nc) as tc:
        with (
            tc.tile_pool(name="const", bufs=1) as cpool,
            tc.tile_pool(name="stage", bufs=4) as opool,
            tc.tile_pool(name="psum", bufs=8, space="PSUM") as ppool,
        ):
            # ---- resident x slab + conv constants ----
            X = cpool.tile([128, cols], F16)
            X3 = X.rearrange("p (h w) -> p h w", w=W)
            wsb = cpool.tile([128, 12, 128], F16)
            nc.sync.dma_start(out=wsb[:], in_=wts[:])
            gsb = cpool.tile([128, 1], F32)
            nc.sync.dma_start(out=gsb[:], in_=bgi[:])
            bsb = cpool.tile([128, 1], F32)
            nc.sync.dma_start(out=bsb[:], in_=cbias[:])
            # dummy Sign: pulls ACT_TABLE_LOAD off the critical path
            warm = cpool.tile([128, 1], F8)
            nc.scalar.activation(out=warm[:], in_=bsb[:],
                                 func=mybir.ActivationFunctionType.Sign)
            # warmup AllReduce on junk data: absorbs the first-collective
            # setup cost / inter-core launch skew (~50us) under the
            # stats read, so the real 512B AllReduce is fast
            nc.gpsimd.collective_compute(
                "AllReduce", mybir.AluOpType.add,
                replica_groups=[list(range(N_CORES))],
                ins=[cc_win.ap()], outs=[cc_wout.ap()],
            )

            # ---- stats accumulators ----
            dstats = cpool.tile([128, N_DVE_GROUPS, 6], F32)
            mv = cpool.tile([128, 6], F32)
            asum = cpool.tile([128, NAC], F32)
            asq = cpool.tile([128, NAC], F32)
            trash = cpool.tile([128, 1024], F16)
            tsb = cpool.tile([128, 1], F32)      # -t per partition

            # ---- strips / tmps ----
            strips = [
                [
                    cpool.tile([128, STRIP_LEN], F8, name=f"strip{im}_{pb}",
                               tag=f"strip{im}_{pb}")
                    for pb in range(2)
                ]
                for im in range(2)
            ]
            for im in range(2):
                for pb in range(2):
                    s3 = strips[im][pb].rearrange("p (s c) -> p s c", c=WP)
                    nc.vector.memset(s3[:, :, 0], 0.0)
                    nc.vector.memset(s3[:, :, WP - 1], 0.0)
                    nc.vector.memset(s3[:, 0, :], 0.0)
                    nc.vector.memset(s3[:, SLOTS - 1, :], 0.0)
            # 6 sign buffers (pads zeroed once -> whole-slot contiguous
            # copies); 2-band rotation keeps the tmp WAR two bands back.
            tmps = [cpool.tile([128, 12, WP], F8, name=f"tmp{i}")
                    for i in range(6)]
            for t3 in tmps:
                nc.vector.memset(t3[:, :, 0], 0.0)
                nc.vector.memset(t3[:, :, WP - 1], 0.0)

            def emit_stats():
                g0 = 0
                di = 0
                ai = 0
                for ng, eng in STATS_CHUNKS:
                    c0, c1 = g0 * 512, (g0 + ng) * 512
                    nc.sync.dma_start(out=X[:, c0:c1], in_=x16[:, c0:c1])
                    if eng == "A":
                        off = c0
                        rem = ng * 512
                        while rem > 0:
                            sz = min(1024, rem)
                            nc.scalar.activation(
                                out=trash[:, 0:sz],
                                in_=X[:, off : off + sz],
                                func=mybir.ActivationFunctionType.Copy,
                                accum_out=asum[:, ai : ai + 1],
                            )
                            nc.scalar.activation(
                                out=trash[:, 0:sz],
                                in_=X[:, off : off + sz],
                                func=mybir.ActivationFunctionType.Square,
                                accum_out=asq[:, ai : ai + 1],
                            )
                            off += sz
                            rem -= sz
                            ai += 1
                    else:
                        for g in range(ng):
                            nc.vector.bn_stats(
                                out=dstats[:, di, :],
                                in_=X[:, c0 + g * 512 : c0 + (g + 1) * 512],
                            )
                            di += 1
                    g0 += ng
                assert ai == NAC and di == N_DVE_GROUPS

                # ---- per-partition sums (f32) ----
                nc.vector.bn_aggr(out=mv[:, 0:2], in_=dstats[:])
                n_dve = float(N_DVE_GROUPS * 512)
                AX = mybir.AxisListType.XYZW
                AD = mybir.AluOpType
                ra = cpool.tile([128, 1], F32, name="ra")
                rq = cpool.tile([128, 1], F32, name="rq")
                nc.vector.tensor_reduce(out=ra[:], in_=asum[:], axis=AX,
                                        op=AD.add)
                nc.vector.tensor_reduce(out=rq[:], in_=asq[:], axis=AX,
                                        op=AD.add)
                sx = cpool.tile([128, 1], F32, name="sx")
                sq = cpool.tile([128, 1], F32, name="sq")
                m2 = cpool.tile([128, 1], F32, name="m2t")
                # sx = mean_dve*n_dve + sum(asum)
                nc.vector.tensor_scalar(out=sx[:], in0=mv[:, 0:1],
                                        scalar1=n_dve, scalar2=None,
                                        op0=AD.mult)
                nc.vector.tensor_tensor(out=sx[:], in0=sx[:], in1=ra[:],
                                        op=AD.add)
                # sq = (var_dve + mean_dve^2)*n_dve + sum(asq)
                nc.vector.tensor_tensor(out=m2[:], in0=mv[:, 0:1],
                                        in1=mv[:, 0:1], op=AD.mult)
                nc.vector.tensor_tensor(out=m2[:], in0=m2[:], in1=mv[:, 1:2],
                                        op=AD.add)
                nc.vector.tensor_scalar(out=sq[:], in0=m2[:],
                                        scalar1=n_dve, scalar2=None,
                                        op0=AD.mult)
                nc.vector.tensor_tensor(out=sq[:], in0=sq[:], in1=rq[:],
                                        op=AD.add)
                # fold image halves: per-channel sums on partitions 0:64
                up = cpool.tile([64, 2], F32, name="up")
                nc.sync.dma_start(out=up[:, 0:1], in_=sx[64:128, :])
                nc.sync.dma_start(out=up[:, 1:2], in_=sq[64:128, :])
                sc = cpool.tile([64, 2], F32, name="sc")
                nc.vector.tensor_tensor(out=sc[:, 0:1], in0=sx[0:64, :],
                                        in1=up[:, 0:1], op=AD.add)
                nc.vector.tensor_tensor(out=sc[:, 1:2], in0=sq[0:64, :],
                                        in1=up[:, 1:2], op=AD.add)
                # ---- 8-core AllReduce of [64,2] channel sums ----
                # (DRAM tensor deps are tracked at the bass level; the
                # collective orders correctly after the cc_in write --
                # verified with a delayed-input probe.)
                nc.sync.dma_start(out=cc_in.ap(), in_=sc[:])
                nc.gpsimd.collective_compute(
                    "AllReduce", AD.add,
                    replica_groups=[list(range(N_CORES))],
                    ins=[cc_in.ap()], outs=[cc_out.ap()],
                )
                gs = cpool.tile([64, 2], F32, name="gs")
                nc.sync.dma_start(out=gs[:], in_=cc_out.ap())
                # ---- threshold: tneg = (bias/gamma)*sqrt(var+eps) - mean
                mean = cpool.tile([64, 1], F32, name="mean")
                ex2 = cpool.tile([64, 1], F32, name="ex2")
                var = cpool.tile([64, 1], F32, name="var")
                sd = cpool.tile([64, 1], F32, name="sd")
                epst = cpool.tile([64, 1], F32, name="epst")
                nc.vector.memset(epst[:], BN_EPS)
                inv_n = 1.0 / float(NTOT)
                nc.vector.tensor_scalar(out=mean[:], in0=gs[:, 0:1],
                                        scalar1=inv_n, scalar2=None,
                                        op0=AD.mult)
                nc.vector.tensor_scalar(out=ex2[:], in0=gs[:, 1:2],
                                        scalar1=inv_n, scalar2=None,
                                        op0=AD.mult)
                nc.vector.tensor_tensor(out=var[:], in0=mean[:], in1=mean[:],
                                        op=AD.mult)
                nc.vector.tensor_tensor(out=var[:], in0=ex2[:], in1=var[:],
                                        op=AD.subtract)
                nc.scalar.activation(out=sd[:], in_=var[:],
                                     func=mybir.ActivationFunctionType.Sqrt,
                                     bias=epst[:])
                nc.vector.tensor_tensor(out=sd[:], in0=gsb[0:64, :],
                                        in1=sd[:], op=AD.mult)
                nc.vector.tensor_tensor(out=tsb[0:64, :], in0=sd[:],
                                        in1=mean[:], op=AD.subtract)
                nc.sync.dma_start(out=tsb[64:128, :], in_=tsb[0:64, :])

            def sign_band(b):
                # binarize 128 wide (both images at once) straight from
                # the resident slab into tmp, then 4 DMA copies build the
                # strips: direct halves at slots [lo,hi), shifted halves
                # at [lo-1, hi-1).  Runs one band ahead of the matmuls.
                r0 = b * BAND
                s0 = 1 if b == 0 else 0
                send = SLOTS if b < NB - 1 else SLOTS - 1
                sA = strips[0][b % 2]
                sB = strips[1][b % 2]
                s3A = sA.rearrange("p (s c) -> p s c", c=WP)
                s3B = sB.rearrange("p (s c) -> p s c", c=WP)

                if b == NB - 1:
                    # shifted halves' bottom pad: slot 28 holds the
                    # (zero) slot-29 data; stale from band NB-3.
                    nc.vector.memset(s3A[64:128, SLOTS - 2, :], 0.0)
                    nc.vector.memset(s3B[0:64, SLOTS - 2, :], 0.0)

                chunks = ((s0, 10), (10, 18), (18, send))
                for ci, (lo, hi) in enumerate(chunks):
                    ns = hi - lo
                    tmp = tmps[(b % 2) * 3 + ci]
                    nc.scalar.activation(
                        out=tmp[:, 0:ns, 1 : 1 + W],
                        in_=X3[:, r0 - 1 + lo : r0 - 1 + hi, :],
                        func=mybir.ActivationFunctionType.Sign,
                        bias=tsb[:],
                    )
                    tlo = max(lo - 1, 0)
                    j0 = tlo - lo + 1
                    nc.scalar.dma_start(
                        out=s3A[0:64, lo:hi, :],
                        in_=tmp[0:64, 0:ns, :],
                    )
                    nc.scalar.dma_start(
                        out=s3A[64:128, tlo : hi - 1, :],
                        in_=tmp[0:64, j0:ns, :],
                    )
                    nc.gpsimd.dma_start(
                        out=s3B[64:128, lo:hi, :],
                        in_=tmp[64:128, 0:ns, :],
                    )
                    nc.gpsimd.dma_start(
                        out=s3B[0:64, tlo : hi - 1, :],
                        in_=tmp[64:128, j0:ns, :],
                    )

            def matmul_band(b):
                r0 = b * BAND
                g4A = strips[0][b % 2].rearrange("p (g t c) -> p g t c",
                                                 t=2, c=WP)
                g4B = strips[1][b % 2].rearrange("p (g t c) -> p g t c",
                                                 t=2, c=WP)
                stgs = {}
                QT = ((0, 4), (4, NU))

                def do_chunk(im, q):
                    g4 = g4A if im == 0 else g4B
                    if im not in stgs:
                        stgs[im] = opool.tile([128, NT, W], F16,
                                              tag=f"stg{im}",
                                              name=f"stg{b}_{im}")
                    stg = stgs[im]
                    ua, ub = QT[q]
                    # weight-outer per half-band: 3-4 live psum banks,
                    # so two chunks fit in the 8 banks and the next
                    # chunk's matmuls overlap this one's evacs.
                    pss = [
                        ppool.tile([128, 2, W], F32, tag="ps",
                                   name=f"ps{b}_{im}_{u}")
                        for u in range(ua, ub)
                    ]
                    for m in range(6):
                        pair, dx = divmod(m, 3)
                        for j, u in enumerate(range(ua, ub)):
                            g = 2 * u + pair  # slot 4u+2*pair, even
                            nc.tensor.matmul(
                                pss[j][:, :, :],
                                wsb[:, im * 6 + m, :],
                                g4[:, g : g + 2, 0, dx : dx + W],
                                start=(m == 0),
                                stop=(m == 5),
                            )
                    for j, u in enumerate(range(ua, ub)):
                        nc.vector.tensor_scalar(
                            out=stg[:, 2 * u : 2 * u + 2, :],
                            in0=pss[j][:, :, :],
                            scalar1=bsb[:],
                            scalar2=None,
                            op0=mybir.AluOpType.add,
                        )
                    h0 = 0 if q == 0 else NT // 2
                    # sync queue is idle in phase 2: it takes the y writes
                    nc.sync.dma_start(
                        out=y[:, im,
                              r0 // 2 + h0 : r0 // 2 + h0 + NT // 2, :],
                        in_=stg[:, h0 : h0 + NT // 2, :],
                    )

                for im, q in ((0, 0), (1, 0), (0, 1), (1, 1)):
                    do_chunk(im, q)

            def emit_all():
                emit_stats()
                # software pipeline: sign+copies(b+1) | matmuls(b)
                sign_band(0)
                for b in range(NB):
                    if b + 1 < NB:
                        sign_band(b + 1)
                    matmul_band(b)

            if repeat == 1:
                emit_all()
            else:
                with tc.For_i(0, repeat, 1):
                    emit_all()
    nc.compile()
    return nc


_cache = {}


def _get(name, builder):
    if name not in _cache:
        _cache[name] = builder()
    return _cache[name]


def _prep_weights(conv_weight, conv_bias, bn_weight, bn_bias):
    # lhsT bank [128, 12, 128]: m = img*6 + pair*3 + dx.
    wts = np.zeros((128, 12, 128), np.float32)
    for im in range(2):
        for pair in range(2):
            for dx in range(3):
                mi = im * 6 + pair * 3 + dx
                for h in range(2):
                    a_slot = h if im == 0 else 1 - h
                    for bcol in range(2):
                        dy = a_slot - bcol + 2 * pair
                        if 0 <= dy <= 2:
                            wts[
                                h * 64 : h * 64 + 64,
                                mi,
                                bcol * 64 : bcol * 64 + 64,
                            ] = conv_weight[:, :, dy, dx].T
    bgi = np.tile(
        (bn_bias.astype(np.float64) / bn_weight.astype(np.float64))
        .astype(np.float32), 2)[:, None]
    cb = np.tile(conv_bias.astype(np.float32), 2)[:, None]
    return wts.astype(np.float16), bgi, cb


def _unshuffle_y(arr, ipc):
    # arr [128, 2, 112, 224] f16: [b*64+oc, im, r2, col] -> [im, oc, 2*r2+b, col]
    a = arr.astype(np.float32).reshape(2, C, 2, HH, W)  # [b, oc, im, r2, col]
    a = a.transpose(2, 1, 3, 0, 4)             # [im, oc, r2, b, col]
    return a.reshape(ipc, C, H, W)


def kernel(x, bn_weight, bn_bias, conv_weight, conv_bias):
    x = np.ascontiguousarray(np.asarray(x), dtype=np.float32)
    bn_weight = np.asarray(bn_weight, dtype=np.float32)
    bn_bias = np.asarray(bn_bias, dtype=np.float32)
    conv_weight = np.asarray(conv_weight, dtype=np.float32)
    conv_bias = np.asarray(conv_bias, dtype=np.float32)

    # the kernel reads x as fp16: halves the HBM-read floor; the only
    # accuracy cost is sign flips where fp16 rounding crosses the BN
    # threshold (~5e-3 rel err on the reference inputs)
    x16 = x.astype(np.float16)
    ipc = N // N_CORES
    wts, bgi, cb = _prep_weights(conv_weight, conv_bias, bn_weight, bn_bias)

    nc_f = _get("fused", build_fused_nc)
    in_maps = [
        {
            "x16": x16[ipc * c : ipc * (c + 1)].reshape(128, H * W),
            "wts": wts,
            "bgi": bgi,
            "cbias": cb,
        }
        for c in range(N_CORES)
    ]
    res = run_bass_kernel_spmd(nc_f, in_maps, list(range(N_CORES))).results
    y = np.concatenate(
        [_unshuffle_y(res[c]["y"], ipc) for c in range(N_CORES)], axis=0
    )
    return y


# revision 31
# speedup vs baseline: 1.3510x; 1.3510x over previous
"""Trainium2 Bass kernel for nn_BinConv2d: BN(train-mode) -> sign -> 3x3 conv.

Two launches on 8 cores, batch-sharded (2 images/core, 128 partitions =
2 img x 64 ch):

  Launch A (stats), engine-split so neither engine is the wall: DVE
    bn_stats takes 66 of the 98 512-elem groups, ACT computes sum(x) /
    sum(x^2) for the other 32 via Copy/Square with accum_out (per-1024
    sub-groups to bound f32 accumulation error).  ACT chunks are placed
    early and a small DVE chunk last so the post-DMA tail is short.
    Host combines both shares in f64, pools across cores, and folds
    BN+sign into one per-channel threshold t_c = mean_c -
    bias_c*sqrt(var_c+eps)/w_c.

  Launch B (conv): per image pair, sign(x) runs 128 partitions wide
    (both images at once) on ACT into a tmp tile, then 4 SBUF->SBUF
    DMA copies (scalar queue for strip A, gpsimd for strip B) build the
    two per-image strips in fp8e4 ({-1,0,1} exact): partitions = 64 ch
    x 2 halves, second half shifted up one row-slot, so an AP
    strip[:, 2k*226+dx] yields rows 2k/2k+1 across the halves.
    Matmuls are double-tile: one instruction computes TWO 2-row tiles
    (moving AP [2, 224] with slot-pair stride), free size 448, psum
    tile [128, 2, 224] = one 2KB bank; 6 matmuls (2 row-pairs x 3 dx)
    accumulate a 4-row tile; 7 such tiles per image per 28-row band.
    Matmuls run weight-outer in half-band chunks (4+3 tiles) so two
    chunks share the 8 psum banks and evacuations (DVE tensor_scalar
    +bias, 448 wide) overlap the next chunk's matmuls.  y is written by
    gpsimd in the parity-split device layout [128, 2, 112, 224] and
    unshuffled on host.
"""

import sys

if "/opt/trn_rl_repo" not in sys.path:
    sys.path.insert(0, "/opt/trn_rl_repo")

import numpy as np

import concourse.bacc as bacc
import concourse.tile as tile
from concourse import mybir
from concourse.bass_utils import run_bass_kernel_spmd

F32 = mybir.dt.float32
F16 = mybir.dt.float16
F8 = mybir.dt.float8e4

N_CORES = 8
N, C, H, W = 16, 64, 224, 224
BN_EPS = 1e-4
BAND = 28              # output rows per band
NB = H // BAND         # 8 bands
WP = W + 2             # padded strip width (226)
NT = BAND // 2         # 14 2-row tiles per band
NU = BAND // 4         # 7 4-row (double) tiles per band
SLOTS = BAND + 2       # 30 strip slots per band
STRIP_LEN = SLOTS * WP
HH = H // 2            # 112


# stats chunking: (n_groups, engine); 'A' chunks go to ACT (sum/sum^2 via
# accum_out), 'D' chunks to DVE bn_stats.  ACT chunks early, small DVE
# chunk last to shorten the post-DMA tail.
STATS_CHUNKS = [(2, "D"), (13, "A"), (8, "D"), (13, "A"), (13, "D"),
                (6, "A"), (13, "D"), (13, "D"), (13, "D"), (4, "D")]
N_DVE_GROUPS = sum(g for g, e in STATS_CHUNKS if e == "D")  # 66
NAC = sum((g * 512 + 1023) // 1024 for g, e in STATS_CHUNKS if e == "A")  # 17


def build_stats_nc(repeat=1):
    """Per-core moments of x_s [128, 50176] f32, split across engines:
    DVE bn_stats for 66 of the 98 512-elem groups -> stats [128, 2]
    (mean, var over the DVE share); ACT computes per-1024-elem sums of x
    and x^2 via accum_out for the other 32 groups -> asum/asq [128, 17].
    The host combines both shares in f64."""
    nc = bacc.Bacc()
    cols = H * W
    x_s = nc.declare_dram_parameter("x_s", [128, cols], F16, isOutput=False)
    stats_out = nc.declare_dram_parameter("stats", [128, 2], F32, isOutput=True)
    asum_out = nc.declare_dram_parameter("asum", [128, NAC], F32, isOutput=True)
    asq_out = nc.declare_dram_parameter("asq", [128, NAC], F32, isOutput=True)

    assert sum(g for g, _ in STATS_CHUNKS) == cols // 512

    with tile.TileContext(nc) as tc:
        with (
            tc.tile_pool(name="xc", bufs=6) as xpool,
            tc.tile_pool(name="acc", bufs=1) as apool,
        ):
            stats = apool.tile([128, N_DVE_GROUPS, 6], F32)
            mv = apool.tile([128, 2], F32)
            asum = apool.tile([128, NAC], F32)
            asq = apool.tile([128, NAC], F32)
            trash = apool.tile([128, 1024], F16)

            def emit_all():
                g0 = 0
                di = 0
                ai = 0
                for ng, eng in STATS_CHUNKS:
                    xt = xpool.tile([128, 13 * 512], F16, tag="xt")
                    nc.sync.dma_start(
                        out=xt[:, : ng * 512],
                        in_=x_s[:, g0 * 512 : (g0 + ng) * 512],
                    )
                    if eng == "A":
                        off = 0
                        rem = ng * 512
                        while rem > 0:
                            sz = min(1024, rem)
                            nc.scalar.activation(
                                out=trash[:, 0:sz],
                                in_=xt[:, off : off + sz],
                                func=mybir.ActivationFunctionType.Copy,
                                accum_out=asum[:, ai : ai + 1],
                            )
                            nc.scalar.activation(
                                out=trash[:, 0:sz],
                                in_=xt[:, off : off + sz],
                                func=mybir.ActivationFunctionType.Square,
                                accum_out=asq[:, ai : ai + 1],
                            )
                            off += sz
                            rem -= sz
                            ai += 1
                    else:
                        for g in range(ng):
                            nc.vector.bn_stats(
                                out=stats[:, di, :],
                                in_=xt[:, g * 512 : (g + 1) * 512],
                            )
                            di += 1
                    g0 += ng
                assert ai == NAC and di == N_DVE_GROUPS
                nc.vector.bn_aggr(out=mv[:], in_=stats[:])
                nc.sync.dma_start(out=stats_out[:], in_=mv[:])
                nc.scalar.dma_start(out=asum_out[:], in_=asum[:])
                nc.scalar.dma_start(out=asq_out[:], in_=asq[:])

            if repeat == 1:
                emit_all()
            else:
                with tc.For_i(0, repeat, 1):
                    emit_all()
    nc.compile()
    return nc


def build_conv_nc(repeat=1):
    """Per-core conv kernel: x16 [128, H*W] f16 (2 img x 64 ch) streamed
    into a resident SBUF slab, wts [128, 12, 128] fp16 lhsT bank,
    tneg [128,1], cbias [128,1] -> y [128, 2, 112, 224] f16."""
    nc = bacc.Bacc()
    cols = H * W
    x16 = nc.declare_dram_parameter("x16", [128, cols], F16, isOutput=False)
    wts = nc.declare_dram_parameter("wts", [128, 12, 128], F16, isOutput=False)
    tneg = nc.declare_dram_parameter("tneg", [128, 1], F32, isOutput=False)
    cbias = nc.declare_dram_parameter("cbias", [128, 1], F32, isOutput=False)
    y = nc.declare_dram_parameter("y", [128, 2, HH, W], F16, isOutput=True)

    with tile.TileContext(nc) as tc:
        with (
            tc.tile_pool(name="const", bufs=1) as cpool,
            tc.tile_pool(name="stage", bufs=4) as opool,
            tc.tile_pool(name="psum", bufs=8, space="PSUM") as ppool,
        ):
            X = cpool.tile([128, cols], F16)
            X3 = X.rearrange("p (h w) -> p h w", w=W)
            wsb = cpool.tile([128, 12, 128], F16)
            nc.sync.dma_start(out=wsb[:], in_=wts[:])
            tsb = cpool.tile([128, 1], F32)
            nc.sync.dma_start(out=tsb[:], in_=tneg[:])
            bsb = cpool.tile([128, 1], F32)
            nc.sync.dma_start(out=bsb[:], in_=cbias[:])
            # dummy Sign: pulls ACT_TABLE_LOAD off the critical path
            warm = cpool.tile([128, 1], F8)
            nc.scalar.activation(out=warm[:], in_=bsb[:],
                                 func=mybir.ActivationFunctionType.Sign)

            strips = [
                [
                    cpool.tile([128, STRIP_LEN], F8, name=f"strip{im}_{pb}",
                               tag=f"strip{im}_{pb}")
                    for pb in range(2)
                ]
                for im in range(2)
            ]
            for im in range(2):
                for pb in range(2):
                    s3 = strips[im][pb].rearrange("p (s c) -> p s c", c=WP)
                    nc.vector.memset(s3[:, :, 0], 0.0)
                    nc.vector.memset(s3[:, :, WP - 1], 0.0)
                    nc.vector.memset(s3[:, 0, :], 0.0)
                    nc.vector.memset(s3[:, SLOTS - 1, :], 0.0)
            # 6 sign buffers (pads zeroed once -> whole-slot contiguous
            # copies); 2-band rotation keeps the tmp WAR two bands back.
            tmps = [cpool.tile([128, 12, WP], F8, name=f"tmp{i}")
                    for i in range(6)]
            for t3 in tmps:
                nc.vector.memset(t3[:, :, 0], 0.0)
                nc.vector.memset(t3[:, :, WP - 1], 0.0)

            def sign_band(b):
                # binarize 128 wide (both images at once) straight from
                # the resident slab into tmp, then 4 DMA copies build the
                # strips: direct halves at slots [lo,hi), shifted halves
                # at [lo-1, hi-1).  Runs one band ahead of the matmuls.
                r0 = b * BAND
                s0 = 1 if b == 0 else 0
                send = SLOTS if b < NB - 1 else SLOTS - 1
                sA = strips[0][b % 2]
                sB = strips[1][b % 2]
                s3A = sA.rearrange("p (s c) -> p s c", c=WP)
                s3B = sB.rearrange("p (s c) -> p s c", c=WP)

                if b == NB - 1:
                    # shifted halves' bottom pad: slot 28 holds the
                    # (zero) slot-29 data; stale from band NB-3.
                    nc.vector.memset(s3A[64:128, SLOTS - 2, :], 0.0)
                    nc.vector.memset(s3B[0:64, SLOTS - 2, :], 0.0)

                chunks = ((s0, 10), (10, 18), (18, send))
                for ci, (lo, hi) in enumerate(chunks):
                    ns = hi - lo
                    tmp = tmps[(b % 2) * 3 + ci]
                    nc.scalar.activation(
                        out=tmp[:, 0:ns, 1 : 1 + W],
                        in_=X3[:, r0 - 1 + lo : r0 - 1 + hi, :],
                        func=mybir.ActivationFunctionType.Sign,
                        bias=tsb[:],
                    )
                    tlo = max(lo - 1, 0)
                    j0 = tlo - lo + 1
                    nc.scalar.dma_start(
                        out=s3A[0:64, lo:hi, :],
                        in_=tmp[0:64, 0:ns, :],
                    )
                    nc.scalar.dma_start(
                        out=s3A[64:128, tlo : hi - 1, :],
                        in_=tmp[0:64, j0:ns, :],
                    )
                    nc.gpsimd.dma_start(
                        out=s3B[64:128, lo:hi, :],
                        in_=tmp[64:128, 0:ns, :],
                    )
                    nc.gpsimd.dma_start(
                        out=s3B[0:64, tlo : hi - 1, :],
                        in_=tmp[64:128, j0:ns, :],
                    )

            def matmul_band(b):
                r0 = b * BAND
                g4A = strips[0][b % 2].rearrange("p (g t c) -> p g t c",
                                                 t=2, c=WP)
                g4B = strips[1][b % 2].rearrange("p (g t c) -> p g t c",
                                                 t=2, c=WP)
                stgs = {}
                QT = ((0, 4), (4, NU))

                def do_chunk(im, q):
                    g4 = g4A if im == 0 else g4B
                    if im not in stgs:
                        stgs[im] = opool.tile([128, NT, W], F16,
                                              tag=f"stg{im}",
                                              name=f"stg{b}_{im}")
                    stg = stgs[im]
                    ua, ub = QT[q]
                    # weight-outer per half-band: 3-4 live psum banks,
                    # so two chunks fit in the 8 banks and the next
                    # chunk's matmuls overlap this one's evacs.
                    pss = [
                        ppool.tile([128, 2, W], F32, tag="ps",
                                   name=f"ps{b}_{im}_{u}")
                        for u in range(ua, ub)
                    ]
                    for m in range(6):
                        pair, dx = divmod(m, 3)
                        for j, u in enumerate(range(ua, ub)):
                            g = 2 * u + pair  # slot 4u+2*pair, even
                            nc.tensor.matmul(
                                pss[j][:, :, :],
                                wsb[:, im * 6 + m, :],
                                g4[:, g : g + 2, 0, dx : dx + W],
                                start=(m == 0),
                                stop=(m == 5),
                            )
                    for j, u in enumerate(range(ua, ub)):
                        nc.vector.tensor_scalar(
                            out=stg[:, 2 * u : 2 * u + 2, :],
                            in0=pss[j][:, :, :],
                            scalar1=bsb[:],
                            scalar2=None,
                            op0=mybir.AluOpType.add,
                        )
                    h0 = 0 if q == 0 else NT // 2
                    nc.gpsimd.dma_start(
                        out=y[:, im,
                              r0 // 2 + h0 : r0 // 2 + h0 + NT // 2, :],
                        in_=stg[:, h0 : h0 + NT // 2, :],
                    )

                for im, q in ((0, 0), (1, 0), (0, 1), (1, 1)):
                    do_chunk(im, q)

            def emit_all():
                # stream the slab in 16 pieces (14 rows each) so band 0's
                # sign starts after ~3 pieces; sign+copies(b+1) pipeline
                # one band ahead of matmuls(b)
                PIECES = 16
                pc = cols // PIECES
                for k in range(PIECES):
                    nc.sync.dma_start(out=X[:, k * pc : (k + 1) * pc],
                                      in_=x16[:, k * pc : (k + 1) * pc])
                sign_band(0)
                for b in range(NB):
                    if b + 1 < NB:
                        sign_band(b + 1)
                    matmul_band(b)

            if repeat == 1:
                emit_all()
            else:
                with tc.For_i(0, repeat, 1):
                    emit_all()
    nc.compile()
    return nc


_cache = {}


def _get(name, builder):
    if name not in _cache:
        _cache[name] = builder()
    return _cache[name]


def _prep_conv_inputs(x, bn_weight, bn_bias, conv_weight, conv_bias, stats):
    # per-core results: DVE share (mean, var over 66*512 elems) + ACT
    # share (17 sums of x and x^2) -> exact per-(img,ch) moments in f64,
    # then pool to per-channel batch stats
    ipc = N // N_CORES
    cols = H * W
    n_dve = N_DVE_GROUPS * 512
    meanM = np.empty((N, C), np.float64)
    varM = np.empty((N, C), np.float64)
    for c in range(N_CORES):
        r = stats[c]
        mv = r["stats"].astype(np.float64)
        tot = mv[:, 0] * n_dve + r["asum"].astype(np.float64).sum(1)
        tot2 = (mv[:, 1] + mv[:, 0] ** 2) * n_dve + r["asq"].astype(
            np.float64
        ).sum(1)
        mean_p = tot / cols
        var_p = tot2 / cols - mean_p**2
        s = np.stack([mean_p, var_p], -1).reshape(ipc, C, 2)
        meanM[ipc * c : ipc * (c + 1)] = s[..., 0]
        varM[ipc * c : ipc * (c + 1)] = s[..., 1]
    m = meanM.mean(axis=0)
    v = (varM + meanM**2).mean(axis=0) - m**2
    t = m - bn_bias.astype(np.float64) * np.sqrt(v + BN_EPS) / bn_weight.astype(
        np.float64
    )
    tneg = np.tile((-t).astype(np.float32), 2)[:, None]  # [128,1]
    cb = np.tile(conv_bias.astype(np.float32), 2)[:, None]

    # lhsT bank [128, 12, 128]: m = img*6 + pair*3 + dx.
    wts = np.zeros((128, 12, 128), np.float32)
    for im in range(2):
        for pair in range(2):
            for dx in range(3):
                mi = im * 6 + pair * 3 + dx
                for h in range(2):
                    a_slot = h if im == 0 else 1 - h
                    for bcol in range(2):
                        dy = a_slot - bcol + 2 * pair
                        if 0 <= dy <= 2:
                            wts[
                                h * 64 : h * 64 + 64,
                                mi,
                                bcol * 64 : bcol * 64 + 64,
                            ] = conv_weight[:, :, dy, dx].T
    return wts.astype(np.float16), tneg, cb


def _unshuffle_y(arr, ipc):
    # arr [128, 2, 112, 224] f16: [b*64+oc, im, r2, col] -> [im, oc, 2*r2+b, col]
    a = arr.astype(np.float32).reshape(2, C, 2, HH, W)  # [b, oc, im, r2, col]
    a = a.transpose(2, 1, 3, 0, 4)             # [im, oc, r2, b, col]
    return a.reshape(ipc, C, H, W)


def kernel(x, bn_weight, bn_bias, conv_weight, conv_bias):
    x = np.ascontiguousarray(np.asarray(x), dtype=np.float32)
    # both launches read x as fp16: halves the HBM-read floor of each
    # launch; the only accuracy cost is sign flips where fp16 rounding
    # crosses the BN threshold (~5e-3 rel err, measured 341 flips of
    # 51.4M elements on the reference inputs)
    x16 = x.astype(np.float16)
    bn_weight = np.asarray(bn_weight, dtype=np.float32)
    bn_bias = np.asarray(bn_bias, dtype=np.float32)
    conv_weight = np.asarray(conv_weight, dtype=np.float32)
    conv_bias = np.asarray(conv_bias, dtype=np.float32)

    ipc = N // N_CORES
    nc_s = _get("stats", build_stats_nc)
    in_maps = [
        {"x_s": x16[ipc * c : ipc * (c + 1)].reshape(128, H * W)}
        for c in range(N_CORES)
    ]
    res = run_bass_kernel_spmd(nc_s, in_maps, list(range(N_CORES))).results
    stats = [res[c] for c in range(N_CORES)]

    wts, tneg, cb = _prep_conv_inputs(
        x, bn_weight, bn_bias, conv_weight, conv_bias, stats
    )

    nc_c = _get("conv", build_conv_nc)
    in_maps = [
        {
            "x16": x16[ipc * c : ipc * (c + 1)].reshape(128, H * W),
            "wts": wts,
            "tneg": tneg,
            "cbias": cb,
        }
        for c in range(N_CORES)
    ]
    res = run_bass_kernel_spmd(nc_c, in_maps, list(range(N_CORES))).results
    y = np.concatenate(
        [_unshuffle_y(res[c]["y"], ipc) for c in range(N_CORES)], axis=0
    )
    return y


# revision 32
# speedup vs baseline: 1.4245x; 1.0544x over previous
"""Trainium2 Bass kernel for nn_BinConv2d: BN(train-mode) -> sign -> 3x3 conv.

Two launches on 8 cores, batch-sharded (2 images/core, 128 partitions =
2 img x 64 ch):

  Launch A (stats), engine-split so neither engine is the wall: DVE
    bn_stats takes 66 of the 98 512-elem groups, ACT computes sum(x) /
    sum(x^2) for the other 32 via Copy/Square with accum_out (per-1024
    sub-groups to bound f32 accumulation error).  ACT chunks are placed
    early and a small DVE chunk last so the post-DMA tail is short.
    Host combines both shares in f64, pools across cores, and folds
    BN+sign into one per-channel threshold t_c = mean_c -
    bias_c*sqrt(var_c+eps)/w_c.

  Launch B (conv): per image pair, sign(x) runs 128 partitions wide
    (both images at once) on ACT into a tmp tile, then 4 SBUF->SBUF
    DMA copies (scalar queue for strip A, gpsimd for strip B) build the
    two per-image strips in fp8e4 ({-1,0,1} exact): partitions = 64 ch
    x 2 halves, second half shifted up one row-slot, so an AP
    strip[:, 2k*226+dx] yields rows 2k/2k+1 across the halves.
    Matmuls are double-tile: one instruction computes TWO 2-row tiles
    (moving AP [2, 224] with slot-pair stride), free size 448, psum
    tile [128, 2, 224] = one 2KB bank; 6 matmuls (2 row-pairs x 3 dx)
    accumulate a 4-row tile; 7 such tiles per image per 28-row band.
    Matmuls run weight-outer in half-band chunks (4+3 tiles) so two
    chunks share the 8 psum banks and evacuations (DVE tensor_scalar
    +bias, 448 wide) overlap the next chunk's matmuls.  y is written by
    gpsimd in the parity-split device layout [128, 2, 112, 224] and
    unshuffled on host.
"""

import sys

if "/opt/trn_rl_repo" not in sys.path:
    sys.path.insert(0, "/opt/trn_rl_repo")

import numpy as np

import concourse.bacc as bacc
import concourse.tile as tile
from concourse import mybir
from concourse.bass_utils import run_bass_kernel_spmd

F32 = mybir.dt.float32
F16 = mybir.dt.float16
F8 = mybir.dt.float8e4

N_CORES = 8
N, C, H, W = 16, 64, 224, 224
BN_EPS = 1e-4
BAND = 28              # output rows per band
NB = H // BAND         # 8 bands
WP = W + 2             # padded strip width (226)
NT = BAND // 2         # 14 2-row tiles per band
NU = BAND // 4         # 7 4-row (double) tiles per band
SLOTS = BAND + 2       # 30 strip slots per band
STRIP_LEN = SLOTS * WP
HH = H // 2            # 112


# stats chunking: (n_groups, engine); 'A' chunks go to ACT (sum/sum^2 via
# accum_out), 'D' chunks to DVE bn_stats.  ACT chunks early, small DVE
# chunk last to shorten the post-DMA tail.
STATS_CHUNKS = [(2, "D"), (13, "A"), (8, "D"), (13, "A"), (13, "D"),
                (6, "A"), (13, "D"), (13, "D"), (13, "D"), (4, "D")]
N_DVE_GROUPS = sum(g for g, e in STATS_CHUNKS if e == "D")  # 66
NAC = sum((g * 512 + 1023) // 1024 for g, e in STATS_CHUNKS if e == "A")  # 17


def build_stats_nc(repeat=1):
    """Per-core moments of x_s [128, 50176] f32, split across engines:
    DVE bn_stats for 66 of the 98 512-elem groups -> stats [128, 2]
    (mean, var over the DVE share); ACT computes per-1024-elem sums of x
    and x^2 via accum_out for the other 32 groups -> asum/asq [128, 17].
    The host combines both shares in f64."""
    nc = bacc.Bacc()
    cols = H * W
    x_s = nc.declare_dram_parameter("x_s", [128, cols], F16, isOutput=False)
    stats_out = nc.declare_dram_parameter("stats", [128, 2], F32, isOutput=True)
    asum_out = nc.declare_dram_parameter("asum", [128, NAC], F32, isOutput=True)
    asq_out = nc.declare_dram_parameter("asq", [128, NAC], F32, isOutput=True)

    assert sum(g for g, _ in STATS_CHUNKS) == cols // 512

    with tile.TileContext(nc) as tc:
        with (
            tc.tile_pool(name="xc", bufs=6) as xpool,
            tc.tile_pool(name="acc", bufs=1) as apool,
        ):
            stats = apool.tile([128, N_DVE_GROUPS, 6], F32)
            mv = apool.tile([128, 2], F32)
            asum = apool.tile([128, NAC], F32)
            asq = apool.tile([128, NAC], F32)
            trash = apool.tile([128, 1024], F16)

            def emit_all():
                g0 = 0
                di = 0
                ai = 0
                for ng, eng in STATS_CHUNKS:
                    xt = xpool.tile([128, 13 * 512], F16, tag="xt")
                    nc.sync.dma_start(
                        out=xt[:, : ng * 512],
                        in_=x_s[:, g0 * 512 : (g0 + ng) * 512],
                    )
                    if eng == "A":
                        off = 0
                        rem = ng * 512
                        while rem > 0:
                            sz = min(1024, rem)
                            nc.scalar.activation(
                                out=trash[:, 0:sz],
                                in_=xt[:, off : off + sz],
                                func=mybir.ActivationFunctionType.Copy,
                                accum_out=asum[:, ai : ai + 1],
                            )
                            nc.scalar.activation(
                                out=trash[:, 0:sz],
                                in_=xt[:, off : off + sz],
                                func=mybir.ActivationFunctionType.Square,
                                accum_out=asq[:, ai : ai + 1],
                            )
                            off += sz
                            rem -= sz
                            ai += 1
                    else:
                        for g in range(ng):
                            nc.vector.bn_stats(
                                out=stats[:, di, :],
                                in_=xt[:, g * 512 : (g + 1) * 512],
                            )
                            di += 1
                    g0 += ng
                assert ai == NAC and di == N_DVE_GROUPS
                nc.vector.bn_aggr(out=mv[:], in_=stats[:])
                nc.sync.dma_start(out=stats_out[:], in_=mv[:])
                nc.scalar.dma_start(out=asum_out[:], in_=asum[:])
                nc.scalar.dma_start(out=asq_out[:], in_=asq[:])

            if repeat == 1:
                emit_all()
            else:
                with tc.For_i(0, repeat, 1):
                    emit_all()
    nc.compile()
    return nc


def build_conv_nc(repeat=1):
    """Per-core conv kernel: x16 [128, H*W] f16 (2 img x 64 ch) streamed
    into a resident SBUF slab, wts [128, 12, 128] fp16 lhsT bank,
    tneg [128,1], cbias [128,1] -> y [128, 2, 112, 224] f16."""
    nc = bacc.Bacc()
    cols = H * W
    x16 = nc.declare_dram_parameter("x16", [128, cols], F16, isOutput=False)
    wts = nc.declare_dram_parameter("wts", [128, 12, 128], F16, isOutput=False)
    tneg = nc.declare_dram_parameter("tneg", [128, 1], F32, isOutput=False)
    cbias = nc.declare_dram_parameter("cbias", [128, 1], F32, isOutput=False)
    y = nc.declare_dram_parameter("y", [128, 2, HH, W], F16, isOutput=True)

    with tile.TileContext(nc) as tc:
        with (
            tc.tile_pool(name="const", bufs=1) as cpool,
            tc.tile_pool(name="stage", bufs=4) as opool,
            tc.tile_pool(name="psum", bufs=8, space="PSUM") as ppool,
        ):
            X = cpool.tile([128, cols], F16)
            X3 = X.rearrange("p (h w) -> p h w", w=W)
            wsb = cpool.tile([128, 12, 128], F16)
            nc.sync.dma_start(out=wsb[:], in_=wts[:])
            tsb = cpool.tile([128, 1], F32)
            nc.sync.dma_start(out=tsb[:], in_=tneg[:])
            bsb = cpool.tile([128, 1], F32)
            nc.sync.dma_start(out=bsb[:], in_=cbias[:])
            # dummy Sign: pulls ACT_TABLE_LOAD off the critical path
            warm = cpool.tile([128, 1], F8)
            nc.scalar.activation(out=warm[:], in_=bsb[:],
                                 func=mybir.ActivationFunctionType.Sign)
            pace = cpool.tile([128, 4], F8)

            strips = [
                [
                    cpool.tile([128, STRIP_LEN], F8, name=f"strip{im}_{pb}",
                               tag=f"strip{im}_{pb}")
                    for pb in range(2)
                ]
                for im in range(2)
            ]
            for im in range(2):
                for pb in range(2):
                    s3 = strips[im][pb].rearrange("p (s c) -> p s c", c=WP)
                    nc.vector.memset(s3[:, :, 0], 0.0)
                    nc.vector.memset(s3[:, :, WP - 1], 0.0)
                    nc.vector.memset(s3[:, 0, :], 0.0)
                    nc.vector.memset(s3[:, SLOTS - 1, :], 0.0)
            # 6 sign buffers (pads zeroed once -> whole-slot contiguous
            # copies); 2-band rotation keeps the tmp WAR two bands back.
            tmps = [cpool.tile([128, 12, WP], F8, name=f"tmp{i}")
                    for i in range(6)]
            for t3 in tmps:
                nc.vector.memset(t3[:, :, 0], 0.0)
                nc.vector.memset(t3[:, :, WP - 1], 0.0)

            def sign_band(b):
                # binarize 128 wide (both images at once) straight from
                # the resident slab into tmp, then 4 DMA copies build the
                # strips: direct halves at slots [lo,hi), shifted halves
                # at [lo-1, hi-1).  Runs one band ahead of the matmuls.
                r0 = b * BAND
                s0 = 1 if b == 0 else 0
                send = SLOTS if b < NB - 1 else SLOTS - 1
                sA = strips[0][b % 2]
                sB = strips[1][b % 2]
                s3A = sA.rearrange("p (s c) -> p s c", c=WP)
                s3B = sB.rearrange("p (s c) -> p s c", c=WP)

                if b == NB - 1:
                    # shifted halves' bottom pad: slot 28 holds the
                    # (zero) slot-29 data; stale from band NB-3.
                    nc.vector.memset(s3A[64:128, SLOTS - 2, :], 0.0)
                    nc.vector.memset(s3B[0:64, SLOTS - 2, :], 0.0)

                chunks = ((s0, 10), (10, 18), (18, send))
                for ci, (lo, hi) in enumerate(chunks):
                    ns = hi - lo
                    tmp = tmps[(b % 2) * 3 + ci]
                    nc.scalar.activation(
                        out=tmp[:, 0:ns, 1 : 1 + W],
                        in_=X3[:, r0 - 1 + lo : r0 - 1 + hi, :],
                        func=mybir.ActivationFunctionType.Sign,
                        bias=tsb[:],
                    )
                    tlo = max(lo - 1, 0)
                    j0 = tlo - lo + 1
                    nc.scalar.dma_start(
                        out=s3A[0:64, lo:hi, :],
                        in_=tmp[0:64, 0:ns, :],
                    )
                    nc.scalar.dma_start(
                        out=s3A[64:128, tlo : hi - 1, :],
                        in_=tmp[0:64, j0:ns, :],
                    )
                    nc.gpsimd.dma_start(
                        out=s3B[64:128, lo:hi, :],
                        in_=tmp[64:128, 0:ns, :],
                    )
                    nc.gpsimd.dma_start(
                        out=s3B[0:64, tlo : hi - 1, :],
                        in_=tmp[64:128, j0:ns, :],
                    )

            def matmul_band(b):
                r0 = b * BAND
                g4A = strips[0][b % 2].rearrange("p (g t c) -> p g t c",
                                                 t=2, c=WP)
                g4B = strips[1][b % 2].rearrange("p (g t c) -> p g t c",
                                                 t=2, c=WP)
                stgs = {}
                QT = ((0, 4), (4, NU))

                def do_chunk(im, q):
                    g4 = g4A if im == 0 else g4B
                    if im not in stgs:
                        stgs[im] = opool.tile([128, NT, W], F16,
                                              tag=f"stg{im}",
                                              name=f"stg{b}_{im}")
                    stg = stgs[im]
                    ua, ub = QT[q]
                    # weight-outer per half-band: 3-4 live psum banks,
                    # so two chunks fit in the 8 banks and the next
                    # chunk's matmuls overlap this one's evacs.
                    pss = [
                        ppool.tile([128, 2, W], F32, tag="ps",
                                   name=f"ps{b}_{im}_{u}")
                        for u in range(ua, ub)
                    ]
                    for m in range(6):
                        pair, dx = divmod(m, 3)
                        for j, u in enumerate(range(ua, ub)):
                            g = 2 * u + pair  # slot 4u+2*pair, even
                            nc.tensor.matmul(
                                pss[j][:, :, :],
                                wsb[:, im * 6 + m, :],
                                g4[:, g : g + 2, 0, dx : dx + W],
                                start=(m == 0),
                                stop=(m == 5),
                            )
                    for j, u in enumerate(range(ua, ub)):
                        nc.vector.tensor_scalar(
                            out=stg[:, 2 * u : 2 * u + 2, :],
                            in0=pss[j][:, :, :],
                            scalar1=bsb[:],
                            scalar2=None,
                            op0=mybir.AluOpType.add,
                        )
                    h0 = 0 if q == 0 else NT // 2
                    # sync queue drains its slab pieces early; it takes
                    # the y writes so the gpsimd drain stays short
                    nc.sync.dma_start(
                        out=y[:, im,
                              r0 // 2 + h0 : r0 // 2 + h0 + NT // 2, :],
                        in_=stg[:, h0 : h0 + NT // 2, :],
                    )

                for im, q in ((0, 0), (1, 0), (0, 1), (1, 1)):
                    do_chunk(im, q)

            def piece(k, pc):
                nc.sync.dma_start(out=X[:, k * pc : (k + 1) * pc],
                                  in_=x16[:, k * pc : (k + 1) * pc])

            def emit_all():
                # stream the slab in 16 pieces (14 rows each): 3 up
                # front (band 0's rows), the rest in two groups paced
                # behind sign(0)/sign(2) (1-packet pace DMA reading the
                # last sign tmp + tile_wait_until so the scheduler keeps
                # the order) -- otherwise the bulk load starves the
                # latency-critical strip copies of the first bands.
                PIECES = 16
                pc = cols // PIECES
                for k in range(3):
                    piece(k, pc)
                sign_band(0)
                with tc.tile_wait_until(0.010):
                    nc.sync.dma_start(out=pace[0:1, :],
                                      in_=tmps[2][0:1, 0, 1:5])
                with tc.tile_wait_until(0.0105):
                    for k in range(3, 9):
                        piece(k, pc)
                for b in range(NB):
                    if b + 1 < NB:
                        sign_band(b + 1)
                    if b == 1:
                        with tc.tile_wait_until(0.030):
                            nc.sync.dma_start(out=pace[0:1, :],
                                              in_=tmps[2][0:1, 0, 1:5])
                        with tc.tile_wait_until(0.0305):
                            for k in range(9, PIECES):
                                piece(k, pc)
                    matmul_band(b)

            if repeat == 1:
                emit_all()
            else:
                with tc.For_i(0, repeat, 1):
                    emit_all()
    nc.compile()
    return nc


_cache = {}


def _get(name, builder):
    if name not in _cache:
        _cache[name] = builder()
    return _cache[name]


def _prep_conv_inputs(x, bn_weight, bn_bias, conv_weight, conv_bias, stats):
    # per-core results: DVE share (mean, var over 66*512 elems) + ACT
    # share (17 sums of x and x^2) -> exact per-(img,ch) moments in f64,
    # then pool to per-channel batch stats
    ipc = N // N_CORES
    cols = H * W
    n_dve = N_DVE_GROUPS * 512
    meanM = np.empty((N, C), np.float64)
    varM = np.empty((N, C), np.float64)
    for c in range(N_CORES):
        r = stats[c]
        mv = r["stats"].astype(np.float64)
        tot = mv[:, 0] * n_dve + r["asum"].astype(np.float64).sum(1)
        tot2 = (mv[:, 1] + mv[:, 0] ** 2) * n_dve + r["asq"].astype(
            np.float64
        ).sum(1)
        mean_p = tot / cols
        var_p = tot2 / cols - mean_p**2
        s = np.stack([mean_p, var_p], -1).reshape(ipc, C, 2)
        meanM[ipc * c : ipc * (c + 1)] = s[..., 0]
        varM[ipc * c : ipc * (c + 1)] = s[..., 1]
    m = meanM.mean(axis=0)
    v = (varM + meanM**2).mean(axis=0) - m**2
    t = m - bn_bias.astype(np.float64) * np.sqrt(v + BN_EPS) / bn_weight.astype(
        np.float64
    )
    tneg = np.tile((-t).astype(np.float32), 2)[:, None]  # [128,1]
    cb = np.tile(conv_bias.astype(np.float32), 2)[:, None]

    # lhsT bank [128, 12, 128]: m = img*6 + pair*3 + dx.
    wts = np.zeros((128, 12, 128), np.float32)
    for im in range(2):
        for pair in range(2):
            for dx in range(3):
                mi = im * 6 + pair * 3 + dx
                for h in range(2):
                    a_slot = h if im == 0 else 1 - h
                    for bcol in range(2):
                        dy = a_slot - bcol + 2 * pair
                        if 0 <= dy <= 2:
                            wts[
                                h * 64 : h * 64 + 64,
                                mi,
                                bcol * 64 : bcol * 64 + 64,
                            ] = conv_weight[:, :, dy, dx].T
    return wts.astype(np.float16), tneg, cb


def _unshuffle_y(arr, ipc):
    # arr [128, 2, 112, 224] f16: [b*64+oc, im, r2, col] -> [im, oc, 2*r2+b, col]
    a = arr.astype(np.float32).reshape(2, C, 2, HH, W)  # [b, oc, im, r2, col]
    a = a.transpose(2, 1, 3, 0, 4)             # [im, oc, r2, b, col]
    return a.reshape(ipc, C, H, W)


def kernel(x, bn_weight, bn_bias, conv_weight, conv_bias):
    x = np.ascontiguousarray(np.asarray(x), dtype=np.float32)
    # both launches read x as fp16: halves the HBM-read floor of each
    # launch; the only accuracy cost is sign flips where fp16 rounding
    # crosses the BN threshold (~5e-3 rel err, measured 341 flips of
    # 51.4M elements on the reference inputs)
    x16 = x.astype(np.float16)
    bn_weight = np.asarray(bn_weight, dtype=np.float32)
    bn_bias = np.asarray(bn_bias, dtype=np.float32)
    conv_weight = np.asarray(conv_weight, dtype=np.float32)
    conv_bias = np.asarray(conv_bias, dtype=np.float32)

    ipc = N // N_CORES
    nc_s = _get("stats", build_stats_nc)
    in_maps = [
        {"x_s": x16[ipc * c : ipc * (c + 1)].reshape(128, H * W)}
        for c in range(N_CORES)
    ]
    res = run_bass_kernel_spmd(nc_s, in_maps, list(range(N_CORES))).results
    stats = [res[c] for c in range(N_CORES)]

    wts, tneg, cb = _prep_conv_inputs(
        x, bn_weight, bn_bias, conv_weight, conv_bias, stats
    )

    nc_c = _get("conv", build_conv_nc)
    in_maps = [
        {
            "x16": x16[ipc * c : ipc * (c + 1)].reshape(128, H * W),
            "wts": wts,
            "tneg": tneg,
            "cbias": cb,
        }
        for c in range(N_CORES)
    ]
    res = run_bass_kernel_spmd(nc_c, in_maps, list(range(N_CORES))).results
    y = np.concatenate(
        [_unshuffle_y(res[c]["y"], ipc) for c in range(N_CORES)], axis=0
    )
    return y
